# revision 23
# baseline (speedup 1.0000x reference)
"""Trainium2 Bass kernel for a 5x5 conv2d (NCHW, pad=2, stride=1).

Problem: X [32,32,128,128] f32, K [64,32,5,5] f32 -> out [32,64,128,128].
Sharding: data-parallel over 8 NeuronCores, 4 images per core.

Per-core mapping (the whole trick):
  The 4 images of the shard occupy the 4 PE row-groups (SBUF partitions
  32g..32g+31 hold image g's 32 input channels). Each conv tap (dy,dx)
  of each image is one K=32 x M=64 matmul whose rhs is an access-pattern
  offset into a zero-padded band of the image held in SBUF. With
  tile_position row+col tiling, 4 images x 2 pixel-blocks = 8 concurrent
  matmuls cover all 16 32x32 PE sub-arrays -> full array utilization
  without replicating any data. The 25 taps accumulate in PSUM.
"""

import numpy as np

import concourse.bass as bass
import concourse.tile as tile
from concourse import bacc, mybir
from concourse.bass_utils import run_bass_kernel_spmd

N_CORES = 8
IMGS = 4          # images per core = PE row groups
C = 32            # input channels
O = 64            # output channels
H = W = 128
KH = KW = 5
PAD = 2
WP = W + 2 * PAD  # 132 padded row length
BANDS = 4
BAND_OUT = H // BANDS       # 32 output rows per band
BAND_IN = BAND_OUT + 2 * PAD  # 36 stored padded rows per band
TAPS = KH * KW    # 25
RT = 4            # output rows per psum tile half (RT*W = 512 = max N)

F32 = mybir.dt.float32
# float32r streams at 1 cycle/row (vs 4 for float32) when N>=256.
MM_DT = mybir.dt.float32r


def _build_nc(reps=1):
    nc = bacc.Bacc("TRN2", target_bir_lowering=False, debug=False)
    X = nc.dram_tensor("X", [IMGS, C, H, W], F32, kind="ExternalInput").ap()
    K = nc.dram_tensor("K", [O, C, KH, KW], F32, kind="ExternalInput").ap()
    # host-supplied zeros for the horizontal pad columns (no engine can
    # write float32r directly; DMA is the only legal f32r writer)
    Z = nc.dram_tensor(
        "Z", [128, BAND_IN, 2 * PAD], F32, kind="ExternalInput"
    ).ap()
    out = nc.dram_tensor("out", [IMGS, O, H, W], F32, kind="ExternalOutput").ap()

    with tile.TileContext(nc) as tc:
        with (
            tc.tile_pool(name="wpool", bufs=1) as wpool,
            tc.tile_pool(name="xpool", bufs=3) as xpool,
            tc.tile_pool(name="opool", bufs=8) as opool,
            tc.tile_pool(name="ppool", bufs=8, space="PSUM") as ppool,
        ):
            # Weights: partition 32g+c holds K[o, c, tap] for image-group g
            # (same copy in each of the 4 partition groups so every PE
            # row-group can load its stationary operand locally).
            wt = wpool.tile([128, TAPS, O], MM_DT)
            ksrc = K.rearrange("o c h w -> c (h w) o").bitcast(MM_DT)
            for g in range(IMGS):
                nc.sync.dma_start(wt[32 * g : 32 * g + 32, :, :], ksrc)

            # center tap first: it covers every output element unclipped, so
            # its start=True clears has_written for the whole psum tile.
            tap_order = [(2, 2)] + [
                (dy, dx)
                for dy in range(KH)
                for dx in range(KW)
                if (dy, dx) != (2, 2)
            ]

            def body():
              for b in range(BANDS):
                y0 = b * BAND_OUT  # first output row; padded rows y0..y0+35
                xb = xpool.tile([128, BAND_IN, WP], MM_DT)
                # stored position p holds real input row y0 + p - PAD
                # (out-of-range rows are left unwritten and never read:
                # every tap matmul is clipped to in-image ranges below)
                p_lo = PAD if b == 0 else 0
                p_hi = BAND_IN - 1 - PAD if b == BANDS - 1 else BAND_IN - 1
                r_lo = y0 + p_lo - PAD
                r_hi = y0 + p_hi - PAD
                nc.sync.dma_start(
                    xb[:, :, 0:PAD], Z[:, :, 0:PAD].bitcast(MM_DT)
                )
                nc.sync.dma_start(
                    xb[:, :, PAD + W : WP], Z[:, :, PAD : 2 * PAD].bitcast(MM_DT)
                )
                for g in range(IMGS):
                    nc.sync.dma_start(
                        xb[32 * g : 32 * g + 32, p_lo : p_hi + 1, PAD : PAD + W],
                        X[g, :, r_lo : r_hi + 1, :].bitcast(MM_DT),
                    )

                # 8 psum rounds per band; round t accumulates output rows
                # y0+4t..+3 for each of the 4 images (4 concurrent psum
                # tiles; fp32r matmuls only support col position 0, so
                # M=64 and no col-half split).
                for t in range(BAND_OUT // RT):
                    pss = [
                        ppool.tile(
                            [O, RT, W], F32, name=f"ps_b{b}_t{t}_g{g}", tag="ps"
                        )
                        for g in range(IMGS)
                    ]
                    ybase = RT * t
                    gy = y0 + ybase
                    for ti, (dy, dx) in enumerate(tap_order):
                        first = ti == 0
                        last = ti == TAPS - 1
                        tap = dy * KW + dx
                        # rows are clipped to in-image range (fp32r dst APs
                        # must keep even inner offset/count, so columns use
                        # the zero-padded full-width window instead of clips)
                        ylo = max(gy, PAD - dy)
                        yhi = min(gy + RT - 1, H + 1 - dy)
                        for g in range(IMGS):
                            lhsT = wt[32 * g : 32 * g + 32, tap, :]
                            rhs = xb[
                                32 * g : 32 * g + 32,
                                ylo - y0 + dy : yhi - y0 + dy + 1,
                                dx : dx + W,
                            ]
                            nc.tensor.matmul(
                                pss[g][
                                    :,
                                    ylo - gy : yhi - gy + 1,
                                    :,
                                ],
                                lhsT,
                                rhs,
                                start=first,
                                stop=last,
                                tile_position=(32 * g, 0),
                            )
                    for g in range(IMGS):
                        ob = opool.tile([O, RT, W], F32)
                        nc.any.tensor_copy(ob[:, :, :], pss[g][:, :, :])
                        nc.sync.dma_start(
                            out[g, :, gy : gy + RT, :],
                            ob[:, :, :],
                        )

            if reps > 1:
                with tc.For_i(0, reps, 1):
                    body()
            else:
                body()
    nc.compile()
    return nc


_CACHE = {}


def _get_nc(reps=1):
    if reps not in _CACHE:
        _CACHE[reps] = _build_nc(reps)
    return _CACHE[reps]


def kernel(X, K):
    X = np.ascontiguousarray(np.asarray(X), dtype=np.float32)
    K = np.ascontiguousarray(np.asarray(K), dtype=np.float32)
    nc = _get_nc()
    per = X.shape[0] // N_CORES
    Z = np.zeros((128, BAND_IN, 2 * PAD), dtype=np.float32)
    in_maps = [
        {"X": np.ascontiguousarray(X[per * i : per * (i + 1)]), "K": K, "Z": Z}
        for i in range(N_CORES)
    ]
    res = run_bass_kernel_spmd(nc, in_maps, list(range(N_CORES))).results
    return np.concatenate([res[i]["out"] for i in range(N_CORES)], axis=0)
